# revision 1
# baseline (speedup 1.0000x reference)
"""Trainium2 Bass kernel for nn_Encoder_83992380441041 (causal linear attention
encoder, last-position readout).

Math (per segment b of T tokens):
    yn   = LayerNorm(x_b) * gamma + beta          (beta == 0 here)
    K    = phi(yn @ Wk.T); V = yn @ Wv.T; q = phi(yn[T-1] @ Wq.T)
    out  = q @ (K.T V) / (q . sum_t K_t + eps)    [only last position matters]
with phi(a) = elu(a)+1 = min(exp(a),1) + relu(a).

Key folds:
  * gamma into the weights (host).
  * centering into the weights (host): x @ (W - 1 s~/d) = (x - mu 1) @ W,
    since 1 @ W = s~ (column sums). So the device never materializes x - mu.
  * 1/sqrt(var+eps) into phi's activation scale / tensor_scalar ops.
So: transpose RAW x on the PE straight after DMA; G = xT.T @ W' gives centered
projections; stats (bn_stats) run concurrently off the critical path and only
feed the phi-time scale r.

Sharding: data-parallel over segments. 64 segments -> 8 cores x 8 segments.
"""

import numpy as np

import concourse.bass as bass
import concourse.tile as tile
from concourse import mybir
from concourse.bass_utils import run_bass_kernel_spmd
from concourse.vector_clock import ScopedClock
import bass_rust

EPS_LN = 1e-5
EPS_DEN = 1e-5

F32 = mybir.dt.float32
AF = mybir.ActivationFunctionType
ALU = mybir.AluOpType

N_CORES = 8
F32R = mybir.dt.float32r
import os as _os
_F32R_MODE = _os.environ.get("KERNEL_F32R", "")


def _r(ap, on):
    return ap.bitcast(F32R) if on else ap



def _patched_drain_and_barrier(self, tick_clock, wait_clock):
    # Stock TileContext exit puts one sem-wait per outstanding proc on a
    # single InstDrain; walrus in this container caps sync waits per
    # instruction. Split them across a chain of drains on the same engine
    # (program order preserved => equivalent).
    nc = self.nc
    drain_inst = nc.sync.drain()
    wait_clock.add_sem_waits(
        drain_inst.ins, ScopedClock({None: tick_clock.global_clock})
    )
    si = drain_inst.ins.sync_info
    if si is not None and si.on_wait is not None and len(si.on_wait) > 1:
        waits = list(si.on_wait)
        si.on_wait = waits[:1]
        for w in waits[1:]:
            d2 = nc.sync.drain()
            si2 = d2.ins.sync_info
            if si2 is None:
                d2.ins.sync_info = bass_rust.SyncInfo(on_wait=[w], on_update=[])
            else:
                si2.on_wait = [w]
    nc.all_engine_barrier()
    assert self.sems is not None
    popped = nc._tile_sem_poison_stack.pop()
    assert popped is self._sem_poison
    nc.clear_and_free_semaphores(list(self.sems.allocated().values()))


tile.TileContext._drain_and_barrier = _patched_drain_and_barrier

_orig_commit = tile.TileContext._commit_instruction
_wsplit_counter = [0]


def _patched_commit_instruction(self, inst, lazy_reg_writes: bool = True):
    # Enforce the per-instruction sync-wait capacity of the walrus in this
    # container (1 for regular instructions, 2 for EventSemaphore) by
    # spilling excess waits onto same-engine NOPs committed just before.
    si = getattr(inst, "sync_info", None)
    if si is not None and si.on_wait:
        cap = 2 if isinstance(inst, mybir.InstEventSemaphore) else 1
        if len(si.on_wait) > cap:
            waits = list(si.on_wait)
            si.on_wait = waits[:cap]
            for w in waits[cap:]:
                _wsplit_counter[0] += 1
                nop = mybir.InstNoOp(
                    name=f"wsplit-{_wsplit_counter[0]}",
                    sync_info=mybir.SyncInfo(on_wait=[w], on_update=[]),
                    bass_nofuse=True,
                    engine=inst.engine,
                )
                _orig_commit(self, nop, lazy_reg_writes=False)
    return _orig_commit(self, inst, lazy_reg_writes=lazy_reg_writes)


tile.TileContext._commit_instruction = _patched_commit_instruction


def _build(n_tok: int, n_seg: int, d: int, f: int):
    """Per-core program. Inputs: x [n_tok,d]; wkv [d,2f]=[Wk~|Wv~].T;
    wq [d,f]=(Wq~).T; ident [128,128]. Output: z [n_seg,f]."""
    P = 128
    assert n_tok % P == 0 and d == P
    n_tiles = n_tok // P
    t_seg = n_tok // n_seg
    assert t_seg % P == 0
    tiles_per_seg = t_seg // P
    f2 = 2 * f
    B = 4                       # tiles per block (DMA / PSUM-bank batch)
    n_blk = n_tiles // B
    assert n_tiles % B == 0 and n_blk % 2 == 0

    nc = bass.Bass()
    x_d = nc.declare_dram_parameter("x", [n_tok, d], F32, isOutput=False)
    # packed [wkv | wq | ident] -> one DMA
    wpack_d = nc.declare_dram_parameter(
        "wpack", [P, f2 + f + 2 * P], F32, isOutput=False
    )
    z_d = nc.declare_dram_parameter("z", [n_seg, f], F32, isOutput=True)

    with tile.TileContext(nc) as tc:
        with (
            tc.tile_pool(name="singles", bufs=1) as singles,
            tc.tile_pool(name="phi", bufs=3) as phip,
            tc.tile_pool(name="sseg", bufs=3) as ssegp,
            tc.tile_pool(name="fin", bufs=1) as finp,
            tc.tile_pool(name="psT", bufs=2, space="PSUM") as psT,
            tc.tile_pool(name="psG", bufs=2, space="PSUM") as psG,
            tc.tile_pool(name="psS", bufs=2, space="PSUM") as psS,
            tc.tile_pool(name="psM", bufs=1, space="PSUM") as psM,
        ):
            # --- persistent buffers ---
            xbig = singles.tile([P, n_tok], F32)
            wpack = singles.tile([P, f2 + f + 2 * P], F32)
            xct_big = singles.tile([P, n_tok], F32)
            # per tile: [K' (phi'd in place) | V | ones] = 2f+1 columns
            kvbig = singles.tile([P, n_tiles * (f2 + 1)], F32)
            bnbig = singles.tile([P, n_tiles, 6], F32)
            mv_big = singles.tile([P, 2 * n_tiles], F32)
            rbig = singles.tile([P, n_tiles], F32)
            eps_s = singles.tile([P, 1], F32)

            # --- DMA triggers: wpack first (ident gates the transposes),
            # then x blocks on alternating trigger queues
            nc.sync.dma_start(out=wpack[:], in_=wpack_d[:])
            xsrc = x_d.rearrange("(n p) d -> p n d", p=P)
            for b in range(n_blk):
                eng = nc.scalar if b % 2 == 0 else nc.sync
                eng.dma_start(
                    out=xbig[:, b * B * P:(b + 1) * B * P],
                    in_=xsrc[:, b * B:(b + 1) * B, :],
                )
            wkv_s = wpack[:, 0:f2]
            wq_s = wpack[:, f2:f2 + f]
            ident_s = wpack[:, f2 + f:f2 + f + P]
            wones = wpack[0:1, f2 + f + P:f2 + f + 2 * P]

            nc.vector.memset(eps_s[:], EPS_LN)
            nc.vector.memset(kvbig[:, f2::(f2 + 1)], 1.0)

            # PE warm-up: the HAM clock gate only counts normal-mode matmul
            # activity; feed it dummy matmuls while DMAs are in flight so the
            # real stream starts at 2.4 GHz, and keep-alives below prevent
            # re-throttle during transpose-mode phases.
            junk = singles.tile([P, P], F32)
            nc.vector.memset(junk[:], 0.0)
            for _ in range(6):
                wps = psM.tile([P, P], F32, tag="m")
                nc.tensor.matmul(
                    wps[:], lhsT=junk[:], rhs=junk[:],
                    start=True, stop=True, skip_group_check=True,
                )

            qstack = finp.tile([P, n_seg], F32)
            ndsb = finp.tile([f + 1, n_seg], F32)
            znum = finp.tile([n_seg, f + 1], F32)
            zden = finp.tile([n_seg, 1], F32)
            zout = finp.tile([n_seg, f], F32)
            eq = finp.tile([P, n_seg], F32)
            sq8 = finp.tile([P, n_seg], F32)
            xlast = finp.tile([n_seg, d], F32)
            bn8 = finp.tile([n_seg, 6], F32)
            mv8 = finp.tile([n_seg, 2], F32)
            r8 = finp.tile([n_seg, 1], F32)

            xview = xbig[:].rearrange("p (n d) -> p n d", d=P)

            # last-token rows for the q-path stats
            nc.sync.dma_start(out=xlast[:], in_=x_d[t_seg - 1::t_seg, :])

            # --- stats (feed only the phi-time scale r); batched Sqrt keeps
            # the ACT table resident for the Exp stream that follows
            for n in range(n_tiles):
                nc.vector.bn_stats(out=bnbig[:, n, :], in_=xview[:, n, :])
                nc.vector.bn_aggr(
                    out=mv_big[:, 2 * n:2 * n + 2], in_=bnbig[:, n, :]
                )
            nc.scalar.activation(
                out=rbig[:], in_=mv_big[:, 1::2],
                func=AF.Sqrt, bias=eps_s[:], scale=1.0,
            )
            nc.vector.reciprocal(out=rbig[:], in_=rbig[:])

            # --- per-block chain: transpose(raw x) -> G -> stats -> phi -> S ---
            s_sbs = []
            for b in range(n_blk):
                b0 = b * B
                # transposes of raw x, 4 tiles into one PSUM bank
                pT = psT.tile([P, B * P], F32)
                for j in range(B):
                    n = b0 + j
                    _t = "t" in _F32R_MODE
                    nc.tensor.matmul(
                        _r(pT[:, j * P:(j + 1) * P], _t),
                        lhsT=_r(xview[:, n, :], _t),
                        rhs=_r(ident_s, _t), is_transpose=True,
                        start=True, stop=True, skip_group_check=True,
                    )
                dst = xct_big[:, b0 * P:(b0 + B) * P]
                nc.scalar.copy(out=dst, in_=pT[:])

                if b == n_blk - 1:
                            # --- q batch (emitted after the last xcT copy) ---
                    nc.vector.bn_stats(out=bn8[:], in_=xlast[:])
                    nc.vector.bn_aggr(out=mv8[:], in_=bn8[:])
                    nc.scalar.activation(
                        out=r8[:], in_=mv8[:, 1:2], func=AF.Sqrt,
                        bias=eps_s[:n_seg, :], scale=1.0,
                    )
                    nc.vector.reciprocal(out=r8[:], in_=r8[:])

                    xq = xct_big[:, t_seg - 1::t_seg]
                    # broadcast r8 [8,1] to all partitions: tiny transpose -> ones-row
                    # matmul (K=1) -> [128, n_seg] in PSUM
                    r8r_ps = psM.tile([1, n_seg], F32, tag="m")
                    nc.tensor.matmul(
                        r8r_ps[:], lhsT=r8[:], rhs=ident_s[0:n_seg, 0:n_seg],
                        is_transpose=True, start=True, stop=True, skip_group_check=True,
                    )
                    r8row = finp.tile([1, n_seg], F32)
                    nc.vector.tensor_copy(out=r8row[:], in_=r8r_ps[:])
                    rfull_ps = psM.tile([P, n_seg], F32, tag="m")
                    nc.tensor.matmul(
                        rfull_ps[:], lhsT=wones, rhs=r8row[:],
                        start=True, stop=True, skip_group_check=True,
                    )
                    xqs = finp.tile([P, n_seg], F32)
                    nc.vector.tensor_tensor(
                        out=xqs[:], in0=xq, in1=rfull_ps[:], op=ALU.mult
                    )
                    # q_pre columns at partitions 0..f and f..2f (same values)
                    qc_ps = psM.tile([P, n_seg], F32, tag="m")
                    nc.tensor.matmul(
                        qc_ps[0:f, :], lhsT=wq_s, rhs=xqs[:],
                        start=True, stop=True, skip_group_check=True,
                    )
                    nc.tensor.matmul(
                        qc_ps[f:2 * f, :], lhsT=wq_s, rhs=xqs[:],
                        start=True, stop=True, skip_group_check=True,
                        tile_position=(0, f),
                    )
                    # phi on both copies at once
                    nc.scalar.activation(out=eq[:], in_=qc_ps[:], func=AF.Exp)
                    nc.vector.tensor_scalar_max(
                        out=sq8[:], in0=qc_ps[:], scalar1=0.0
                    )
                    q2big = finp.tile([P, n_seg], F32)
                    nc.vector.scalar_tensor_tensor(
                        out=q2big[:], in0=eq[:], scalar=1.0, in1=sq8[:],
                        op0=ALU.min, op1=ALU.add,
                    )
                    # qstack col 2b = (q_{2b}; 0), col 2b+1 = (0; q_{2b+1})
                    nc.vector.memset(qstack[:], 0.0)
                    nc.vector.tensor_copy(
                        out=qstack[0:f, 0:n_seg:2], in_=q2big[0:f, 0:n_seg:2]
                    )
                    nc.vector.tensor_copy(
                        out=qstack[f:2 * f, 1:n_seg:2], in_=q2big[f:2 * f, 1:n_seg:2]
                    )


                # G = x @ W' (centered via weight fold), 4 tiles per PSUM bank
                gT = psG.tile([P, B * f2], F32)
                for j in range(B):
                    n = b0 + j
                    _g = "g" in _F32R_MODE
                    nc.tensor.matmul(
                        gT[:, j * f2:(j + 1) * f2],
                        lhsT=_r(xct_big[:, n * P:(n + 1) * P], _g),
                        rhs=_r(wkv_s, _g),
                        start=True, stop=True, skip_group_check=True,
                    )

                # one fused r-scale per tile: kv = r * G  ([K_pre | V] at once)
                c = f2 + 1
                for j in range(B):
                    n = b0 + j
                    rcol = rbig[:, n:n + 1]
                    kvdst = kvbig[:, n * c:n * c + f2]
                    if j % 2 == 0:
                        nc.vector.tensor_scalar_mul(
                            out=kvdst, in0=gT[:, j * f2:(j + 1) * f2],
                            scalar1=rcol,
                        )
                    else:
                        nc.scalar.mul(
                            out=kvdst, in_=gT[:, j * f2:(j + 1) * f2],
                            mul=rcol,
                        )
                # batched phi on the K halves, in place
                kv_blk = kvbig[:, b0 * c:(b0 + B) * c].rearrange(
                    "p (j e) -> p j e", e=c
                )
                kh = kv_blk[:, :, 0:f]
                e_t = phip.tile([P, B * f], F32, tag="e")
                nc.scalar.activation(out=e_t[:], in_=kh, func=AF.Exp)
                s_t = phip.tile([P, B * f], F32, tag="s")
                nc.vector.tensor_scalar_max(out=s_t[:], in0=kh, scalar1=0.0)
                nc.vector.scalar_tensor_tensor(
                    out=kh, in0=e_t[:], scalar=1.0, in1=s_t[:],
                    op0=ALU.min, op1=ALU.add,
                )

                # S|Z for the block's two segments, column-packed in the PE
                assert B == 2 * tiles_per_seg
                s_ps = psS.tile([P, f + 1], F32)
                for hh in range(2):
                    s = 2 * b + hh
                    for j in range(tiles_per_seg):
                        n = s * tiles_per_seg + j
                        _s = "s" in _F32R_MODE
                        nc.tensor.matmul(
                            s_ps[hh * f:(hh + 1) * f, :],
                            lhsT=_r(kvbig[:, n * c:n * c + f], _s),
                            rhs=_r(kvbig[:, n * c + f:(n + 1) * c], _s),
                            start=(j == 0), stop=(j == tiles_per_seg - 1),
                            skip_group_check=True,
                            tile_position=(0, hh * f),
                        )
                s_sb = ssegp.tile([P, f + 1], F32)
                nc.scalar.copy(out=s_sb[:], in_=s_ps[:])
                s_sbs.append(s_sb)

            ndT = psM.tile([f + 1, n_seg], F32, tag="nd")
            for b in range(n_blk):
                nc.tensor.matmul(
                    ndT[:, 2 * b:2 * b + 2], lhsT=s_sbs[b][:],
                    rhs=qstack[:, 2 * b:2 * b + 2],
                    start=True, stop=True, skip_group_check=True,
                )

            nc.vector.tensor_copy(out=ndsb[:], in_=ndT[:])
            nd_ps = psM.tile([n_seg, f + 1], F32, tag="m")
            nc.tensor.transpose(nd_ps[:], ndsb[:], ident_s[0:f + 1, 0:f + 1])
            nc.vector.tensor_copy(out=znum[:], in_=nd_ps[:])
            nc.vector.tensor_scalar_add(
                out=zden[:], in0=znum[:, f:f + 1], scalar1=EPS_DEN
            )
            nc.vector.reciprocal(out=zden[:], in_=zden[:])
            nc.vector.tensor_scalar_mul(
                out=zout[:], in0=znum[:, :f], scalar1=zden[:]
            )
            nc.sync.dma_start(out=z_d[:], in_=zout[:])

    return nc


def _prep(inputs):
    x = np.ascontiguousarray(np.asarray(inputs["x"], dtype=np.float32))
    batch = np.asarray(inputs["batch"]).astype(np.int64)
    gamma = np.asarray(inputs["gamma"], dtype=np.float32)
    beta = np.asarray(inputs["beta"], dtype=np.float32)
    wk = np.asarray(inputs["Wk"], dtype=np.float32)
    wq = np.asarray(inputs["Wq"], dtype=np.float32)
    wv = np.asarray(inputs["Wv"], dtype=np.float32)
    n_batches = int(np.asarray(inputs["n_batches"]))

    n, d = x.shape
    f = wk.shape[0]
    t_seg = n // n_batches
    counts = np.bincount(batch, minlength=n_batches)
    if not (np.all(counts == t_seg) and np.all(np.diff(batch) >= 0)):
        raise NotImplementedError("kernel specialized for equal sorted segments")
    if np.any(beta != 0.0):
        raise NotImplementedError("kernel specialized for beta == 0")

    wkg = (wk * gamma[None, :]).astype(np.float64)
    wvg = (wv * gamma[None, :]).astype(np.float64)
    wqg = (wq * gamma[None, :]).astype(np.float64)
    wkv_t = np.concatenate([wkg, wvg], axis=0).T            # [d, 2f]
    wq_t = wqg.T                                            # [d, f]
    # fold the LN centering into the weights:
    #   x @ (W - 1 s~/d) = (x - mu 1) @ W   since 1 @ W = colsums(W)
    wkv_t = wkv_t - wkv_t.sum(axis=0, keepdims=True) / d
    wq_t = wq_t - wq_t.sum(axis=0, keepdims=True) / d
    ident = np.eye(128, dtype=np.float64)
    onesrow = np.zeros((d, 128), dtype=np.float64)
    onesrow[0, :] = 1.0
    wpack = np.ascontiguousarray(
        np.concatenate([wkv_t, wq_t, ident, onesrow], axis=1).astype(np.float32)
    )

    return x, wpack, n, d, f, n_batches, t_seg


def _run(inputs, trace=False):
    x, wpack, n, d, f, n_batches, t_seg = _prep(inputs)

    segs_per_core = n_batches // N_CORES
    tok_per_core = segs_per_core * t_seg
    nc = _build(tok_per_core, segs_per_core, d, f)

    in_maps = []
    for c in range(N_CORES):
        m = {
            "x": np.ascontiguousarray(x[c * tok_per_core:(c + 1) * tok_per_core]),
            "wpack": wpack,
        }
        in_maps.append(m)

    res = run_bass_kernel_spmd(nc, in_maps, list(range(N_CORES)), trace=trace)
    z = np.concatenate([res.results[c]["z"] for c in range(N_CORES)], axis=0)
    return z, res


def kernel(**inputs) -> np.ndarray:
    z, _ = _run(inputs, trace=False)
    return z



# revision 29
# speedup vs baseline: 1.0726x; 1.0726x over previous
"""Trainium2 Bass kernel for nn_Encoder_83992380441041 (causal linear attention
encoder, last-position readout).

Math (per segment b of T tokens):
    yn   = LayerNorm(x_b) * gamma + beta          (beta == 0 here)
    K    = phi(yn @ Wk.T); V = yn @ Wv.T; q = phi(yn[T-1] @ Wq.T)
    out  = q @ (K.T V) / (q . sum_t K_t + eps)    [only last position matters]
with phi(a) = elu(a)+1 = min(exp(a),1) + relu(a).

Key folds:
  * gamma into the weights (host).
  * centering into the weights (host): x @ (W - 1 s~/d) = (x - mu 1) @ W.
  * 1/sqrt(var+eps) via ACT ln/exp (r = exp(-0.5 ln(var128/128+eps))) so the
    ACT table (natural_log_exp) is loaded exactly once; no Sqrt table switch.
  * r applied to RAW x in [token, d] layout (r is per-partition there), an
    SBUF->SBUF op gpsimd can run; everything downstream is pre-normalized.
  * bf16 for the transposed x, the weights and K/V (PE at 1 cyc/row).
  * gpsimd does all SBUF-side elementwise (it cannot touch PSUM); DVE/ACT
    split the PSUM->SBUF extraction.

Pipeline: 5 DMA blocks of [2,4,4,4,2] tiles; per block:
  bn_stats (DVE) -> var combine (gpsimd) -> r (ACT ln/exp) -> xs = r*x
  (gpsimd) -> transposes (PE) -> xct bf16 (DVE); G matmuls (PE bf16) ->
  K copy (DVE) | V copy (ACT) -> phi exp (ACT) + relu/combine (gpsimd)
  -> S|Z matmuls (PE bf16, 2 segs column-packed).
Readout: q from xct columns (already normalized), S.q via PE.

Sharding: data-parallel over segments. 64 segments -> 8 cores x 8 segments.
"""

import numpy as np

import concourse.bass as bass
import concourse.tile as tile
from concourse import mybir
from concourse.bass_utils import run_bass_kernel_spmd
from concourse.vector_clock import ScopedClock
import bass_rust

EPS_LN = 1e-5
EPS_DEN = 1e-5

F32 = mybir.dt.float32
BF16 = mybir.dt.bfloat16
F32R = mybir.dt.float32r
AF = mybir.ActivationFunctionType
ALU = mybir.AluOpType

N_CORES = 8

import os as _os
# t = f32r transposes (1.5 cyc/row vs 2.0 for f32)
_F32R_MODE = _os.environ.get("KERNEL_F32R", "")


def _r(ap, on):
    return ap.bitcast(F32R) if on else ap


def _patched_drain_and_barrier(self, tick_clock, wait_clock):
    # Stock TileContext exit puts one sem-wait per outstanding proc on a
    # single InstDrain; walrus in this container caps sync waits per
    # instruction. Split them across a chain of drains on the same engine
    # (program order preserved => equivalent).
    nc = self.nc
    drain_inst = nc.sync.drain()
    wait_clock.add_sem_waits(
        drain_inst.ins, ScopedClock({None: tick_clock.global_clock})
    )
    si = drain_inst.ins.sync_info
    if si is not None and si.on_wait is not None and len(si.on_wait) > 1:
        waits = list(si.on_wait)
        si.on_wait = waits[:1]
        engines = [nc.sync, nc.scalar, nc.vector, nc.gpsimd, nc.tensor]
        for i, w in enumerate(waits[1:]):
            d2 = engines[(i + 1) % len(engines)].drain()
            si2 = d2.ins.sync_info
            if si2 is None:
                d2.ins.sync_info = bass_rust.SyncInfo(on_wait=[w], on_update=[])
            else:
                si2.on_wait = [w]
    nc.all_engine_barrier()
    assert self.sems is not None
    popped = nc._tile_sem_poison_stack.pop()
    assert popped is self._sem_poison
    nc.clear_and_free_semaphores(list(self.sems.allocated().values()))


tile.TileContext._drain_and_barrier = _patched_drain_and_barrier

_orig_commit = tile.TileContext._commit_instruction
_wsplit_counter = [0]


def _patched_commit_instruction(self, inst, lazy_reg_writes: bool = True):
    # Enforce the per-instruction sync-wait capacity of the walrus in this
    # container (1 for regular instructions, 2 for EventSemaphore) by
    # spilling excess waits onto same-engine NOPs committed just before.
    si = getattr(inst, "sync_info", None)
    if si is not None and si.on_wait:
        cap = 2 if isinstance(inst, mybir.InstEventSemaphore) else 1
        if len(si.on_wait) > cap:
            waits = list(si.on_wait)
            si.on_wait = waits[:cap]
            for w in waits[cap:]:
                _wsplit_counter[0] += 1
                nop = mybir.InstNoOp(
                    name=f"wsplit-{_wsplit_counter[0]}",
                    sync_info=mybir.SyncInfo(on_wait=[w], on_update=[]),
                    bass_nofuse=True,
                    engine=inst.engine,
                )
                _orig_commit(self, nop, lazy_reg_writes=False)
    return _orig_commit(self, inst, lazy_reg_writes=lazy_reg_writes)


tile.TileContext._commit_instruction = _patched_commit_instruction


# tiles per DMA/compute block: small first (earliest pipeline start) and
# small last (short tail chain after the final DMA byte lands)
BLOCKS = tuple(
    int(v) for v in _os.environ.get("KERNEL_BLOCKS", "2,4,4,4,2").split(",")
)
JUNK = tuple(
    int(v) for v in _os.environ.get("KERNEL_JUNK", "16,64,256").split(",") if v
)


def _build(n_tok: int, n_seg: int, d: int, f: int):
    """Per-core program. Inputs: x [n_tok,d]; wpack [128, 320] =
    [wkv~ | wq~ | ident] (weights f32, downcast on device).
    Output: z [n_seg,f]."""
    P = 128
    assert n_tok % P == 0 and d == P
    n_tiles = n_tok // P
    t_seg = n_tok // n_seg
    assert t_seg == 2 * P  # 2 tiles per segment
    f2 = 2 * f
    assert sum(BLOCKS) == n_tiles
    n_blk = len(BLOCKS)
    bounds = [0]
    for bb in BLOCKS:
        bounds.append(bounds[-1] + bb)

    nc = bass.Bass()
    x_d = nc.declare_dram_parameter("x", [n_tok, d], F32, isOutput=False)
    wpack_d = nc.declare_dram_parameter("wpack", [P, f2 + f + P], F32,
                                        isOutput=False)
    z_d = nc.declare_dram_parameter("z", [n_seg, f], F32, isOutput=True)

    with tile.TileContext(nc) as tc:
        with (
            tc.tile_pool(name="singles", bufs=1) as singles,
            tc.tile_pool(name="phi", bufs=3) as phip,
            tc.tile_pool(name="sseg", bufs=1) as ssegp,
            tc.tile_pool(name="fin", bufs=1) as finp,
            tc.tile_pool(name="psT", bufs=2, space="PSUM") as psT,
            tc.tile_pool(name="psG", bufs=2, space="PSUM") as psG,
            tc.tile_pool(name="psS", bufs=2, space="PSUM") as psS,
            tc.tile_pool(name="psM", bufs=1, space="PSUM") as psM,
        ):
            # --- persistent buffers ---
            xbig = singles.tile([P, n_tok], F32)
            wpack = singles.tile([P, f2 + f + P], F32)
            wbf = singles.tile([P, f2 + f], BF16)
            identb = singles.tile([P, P], BF16)
            onecol = singles.tile([P, 1], BF16)
            xct = singles.tile([P, n_tok], BF16)
            kbig = singles.tile([P, n_tiles * f], BF16)
            vbig = singles.tile([P, n_tiles * f], BF16)
            bnb = singles.tile([P, n_tiles, 6], F32)
            varb = singles.tile([P, n_tiles], F32)
            scr = singles.tile([P, n_tiles, 2], F32)
            rbig = singles.tile([P, n_tiles], F32)
            eps_s = singles.tile([P, 1], F32)
            c32 = singles.tile([P, 1], F32)
            junk = singles.tile([P, 256], F32)
            lnj = singles.tile([1, 1], F32)

            # --- DMA triggers: x blocks staggered on sync (in consumption
            # order), wpack on scalar in parallel
            nc.scalar.dma_start(out=wpack[:], in_=wpack_d[:])
            xsrc = x_d.rearrange("(n p) d -> p n d", p=P)
            for b in range(n_blk):
                lo, hi = bounds[b], bounds[b + 1]
                nc.sync.dma_start(
                    out=xbig[:, lo * P:hi * P],
                    in_=xsrc[:, lo:hi, :],
                )

            # --- constants + ACT table preload (overlapped with DMA wait)
            nc.vector.memset(junk[:], 1.0)
            nc.vector.memset(eps_s[:], EPS_LN)
            nc.vector.memset(onecol[:], 1.0)
            nc.vector.memset(c32[:], 32.0)
            # loads the natural_log_exp table once; Ln+Exp+Copy stay resident
            nc.scalar.activation(out=lnj[:], in_=junk[0:1, 0:1], func=AF.Ln)

            # PE warm-up: keep the HAM clock gate fed while DMAs are in
            # flight so the real stream runs at speed.
            pm = psM.tile([P, 512], F32, tag="m")
            for s in JUNK:
                nc.tensor.matmul(
                    pm[0:1, 0:s], lhsT=junk[:, 0:1], rhs=junk[:, 0:s],
                    start=True, stop=True, skip_group_check=True,
                )

            wkv_bf = wbf[:, 0:f2]
            wq_bf = wbf[:, f2:f2 + f]
            ident = wpack[:, f2 + f:f2 + f + P]
            nc.vector.tensor_copy(out=wbf[:], in_=wpack[:, 0:f2 + f])
            nc.vector.tensor_copy(out=identb[:], in_=ident)

            xview = xbig[:].rearrange("p (n d) -> p n d", d=P)

            pend = [None] * n_blk   # (e_t, s_t) per block
            s_sbs = [None] * n_blk

            def emit_relu(b):
                lo, hi = bounds[b], bounds[b + 1]
                gK, e_t, s_t = pend[b]
                nc.scalar.activation(out=s_t[:], in_=gK, func=AF.Relu)

            def emit_stt(b):
                lo, hi = bounds[b], bounds[b + 1]
                gK, e_t, s_t = pend[b]
                nc.vector.scalar_tensor_tensor(
                    out=kbig[:, lo * f:hi * f], in0=e_t[:], scalar=1.0,
                    in1=s_t[:], op0=ALU.min, op1=ALU.add,
                )

            def emit_S(b):
                lo, hi = bounds[b], bounds[b + 1]
                segs = list(range(lo // 2, hi // 2))
                s_ps = psS.tile([P, f + 1], F32, name="s_ps", tag="s")
                for s in segs:
                    hh = s % 2
                    for j in range(2):
                        n = 2 * s + j
                        nc.tensor.matmul(
                            s_ps[hh * f:(hh + 1) * f, 0:f],
                            lhsT=kbig[:, n * f:(n + 1) * f],
                            rhs=vbig[:, n * f:(n + 1) * f],
                            start=(j == 0), stop=(j == 1),
                            skip_group_check=True,
                            tile_position=(0, hh * f),
                        )
                    for j in range(2):
                        n = 2 * s + j
                        nc.tensor.matmul(
                            s_ps[hh * f:(hh + 1) * f, f:f + 1],
                            lhsT=kbig[:, n * f:(n + 1) * f],
                            rhs=onecol[:],
                            start=(j == 0), stop=(j == 1),
                            skip_group_check=True,
                            tile_position=(0, hh * f),
                        )
                s_sb = ssegp.tile([P, f + 1], F32, name="s_sb", tag=f"sb{b}")
                if len(segs) == 1:
                    hh = segs[0] % 2
                    rows = slice(hh * f, (hh + 1) * f)
                else:
                    rows = slice(0, P)
                if b % 2 == 0:
                    nc.vector.tensor_copy(out=s_sb[rows, :], in_=s_ps[rows, :])
                else:
                    nc.scalar.copy(out=s_sb[rows, :], in_=s_ps[rows, :])
                s_sbs[b] = (s_sb, rows)

            for b in range(n_blk):
                lo, hi = bounds[b], bounds[b + 1]
                nb = hi - lo

                # prev block: relu (DVE, reads PSUM) first so its phi chain
                # finishes; then per-tile stats on raw x
                if b > 0:
                    emit_relu(b - 1)
                for j in range(nb):
                    nc.vector.bn_stats(
                        out=bnb[:, lo + j, :], in_=xview[:, lo + j, :]
                    )
                # combine even/odd windows on gpsimd (SBUF only):
                # var*128 = M2e + M2o + 32 (me - mo)^2
                me = bnb[:, lo:hi, 1]
                mo = bnb[:, lo:hi, 4]
                m2e = bnb[:, lo:hi, 2]
                m2o = bnb[:, lo:hi, 5]
                d0 = scr[:, lo:hi, 0]
                s0 = scr[:, lo:hi, 1]
                nc.gpsimd.tensor_tensor(out=d0, in0=me, in1=mo, op=ALU.subtract)
                nc.gpsimd.tensor_tensor(out=d0, in0=d0, in1=d0, op=ALU.mult)
                nc.gpsimd.tensor_tensor(out=s0, in0=m2e, in1=m2o, op=ALU.add)
                nc.gpsimd.tensor_tensor(
                    out=d0, in0=d0,
                    in1=c32[:].broadcast_to((P, hi - lo)), op=ALU.mult,
                )
                nc.gpsimd.tensor_tensor(
                    out=varb[:, lo:hi], in0=d0, in1=s0, op=ALU.add,
                )
                # r = exp(-0.5 ln(var128/128 + eps)) on ACT (table resident)
                nc.scalar.activation(
                    out=rbig[:, lo:hi], in_=varb[:, lo:hi],
                    func=AF.Ln, bias=eps_s[:], scale=1.0 / P,
                )
                nc.scalar.activation(
                    out=rbig[:, lo:hi], in_=rbig[:, lo:hi],
                    func=AF.Exp, scale=-0.5,
                )

                # xs = r * x (bf16) in raw [token, d] layout: r is
                # per-partition here, so gpsimd can do it (SBUF only)
                xs = phip.tile([P, nb * P], BF16, tag="xs")
                for j in range(nb):
                    rb = rbig[:, lo + j:lo + j + 1].broadcast_to((P, P))
                    nc.gpsimd.tensor_tensor(
                        out=xs[:, j * P:(j + 1) * P],
                        in0=xview[:, lo + j, :],
                        in1=rb, op=ALU.mult,
                    )
                # prev block's phi combine + S (stt emitted before S reads K)
                if b > 0:
                    emit_stt(b - 1)
                    emit_S(b - 1)

                # bf16 transposes of the normalized rows (1 cyc/row)
                pT = psT.tile([P, nb * P], BF16, name="pT", tag="t")
                for j in range(nb):
                    nc.tensor.matmul(
                        pT[:, j * P:(j + 1) * P],
                        lhsT=xs[:, j * P:(j + 1) * P],
                        rhs=identb[:], is_transpose=True,
                        start=True, stop=True, skip_group_check=True,
                    )
                # PSUM -> SBUF (DVE, bf16)
                nc.vector.tensor_copy(out=xct[:, lo * P:hi * P], in_=pT[:])

                # G split into K / V halves so downstream APs are contiguous
                gT = psG.tile([P, nb * f2], F32, name="gT", tag="g")
                gK = gT[:, 0:nb * f]
                gV = gT[:, nb * f:nb * f2]
                for j in range(nb):
                    nc.tensor.matmul(
                        gK[:, j * f:(j + 1) * f],
                        lhsT=xct[:, (lo + j) * P:(lo + j + 1) * P],
                        rhs=wkv_bf[:, 0:f],
                        start=True, stop=True, skip_group_check=True,
                    )
                    nc.tensor.matmul(
                        gV[:, j * f:(j + 1) * f],
                        lhsT=xct[:, (lo + j) * P:(lo + j + 1) * P],
                        rhs=wkv_bf[:, f:f2],
                        start=True, stop=True, skip_group_check=True,
                    )

                # phi: e on ACT; V extraction on ACT; relu/stt deferred
                e_t = phip.tile([P, nb * f], BF16, tag="e")
                s_t = phip.tile([P, nb * f], BF16, tag="s")
                nc.scalar.activation(out=e_t[:], in_=gK, func=AF.Exp)
                if b % 2 == 0:
                    nc.vector.tensor_copy(out=vbig[:, lo * f:hi * f], in_=gV)
                else:
                    nc.scalar.copy(out=vbig[:, lo * f:hi * f], in_=gV)
                pend[b] = (gK, e_t, s_t)
                if b == n_blk - 1:
                    emit_relu(b)
                    emit_stt(b)
                    emit_S(b)

            # --- readout: q from xct columns (already normalized) ---
            qstack = finp.tile([P, n_seg], F32)
            ndsb = finp.tile([f + 1, n_seg], F32)
            zden = finp.tile([n_seg, 1], F32)
            zout = finp.tile([n_seg, f], F32)
            eq = finp.tile([P, n_seg], F32)
            sq8 = finp.tile([P, n_seg], F32)
            q2big = finp.tile([P, n_seg], F32)

            xq = xct[:, t_seg - 1::t_seg]
            qc_ps = pm[:, 272:272 + n_seg]
            nc.tensor.matmul(
                qc_ps[0:f, :], lhsT=wq_bf, rhs=xq,
                start=True, stop=True, skip_group_check=True,
            )
            nc.tensor.matmul(
                qc_ps[f:2 * f, :], lhsT=wq_bf, rhs=xq,
                start=True, stop=True, skip_group_check=True,
                tile_position=(0, f),
            )
            nc.scalar.activation(out=eq[:], in_=qc_ps, func=AF.Exp)
            nc.vector.tensor_scalar_max(out=sq8[:], in0=qc_ps, scalar1=0.0)
            nc.vector.scalar_tensor_tensor(
                out=q2big[:], in0=eq[:], scalar=1.0, in1=sq8[:],
                op0=ALU.min, op1=ALU.add,
            )
            # qstack col s: q on the (s%2) partition half, zero on the other
            nc.vector.memset(qstack[:], 0.0)
            nc.vector.tensor_copy(
                out=qstack[0:f, 0:n_seg:2], in_=q2big[0:f, 0:n_seg:2]
            )
            nc.vector.tensor_copy(
                out=qstack[f:2 * f, 1:n_seg:2], in_=q2big[f:2 * f, 1:n_seg:2]
            )

            ndT = pm[0:f + 1, 288:288 + n_seg]
            for b in range(n_blk):
                s0, s1 = bounds[b] // 2, bounds[b + 1] // 2
                s_sb, rows = s_sbs[b]
                nc.tensor.matmul(
                    ndT[:, s0:s1], lhsT=s_sb[rows, :],
                    rhs=qstack[rows, s0:s1],
                    start=True, stop=True, skip_group_check=True,
                )

            nc.vector.tensor_copy(out=ndsb[:], in_=ndT)
            nd_ps = pm[0:n_seg, 304:304 + f + 1]
            nc.tensor.transpose(nd_ps, ndsb[:], ident[0:f + 1, 0:f + 1])
            nc.vector.tensor_scalar_add(
                out=zden[:], in0=nd_ps[:, f:f + 1], scalar1=EPS_DEN
            )
            nc.vector.reciprocal(out=zden[:], in_=zden[:])
            nc.vector.tensor_scalar_mul(
                out=zout[:], in0=nd_ps[:, 0:f], scalar1=zden[:]
            )
            nc.sync.dma_start(out=z_d[:], in_=zout[:])

    return nc


def _prep(inputs):
    x = np.ascontiguousarray(np.asarray(inputs["x"], dtype=np.float32))
    batch = np.asarray(inputs["batch"]).astype(np.int64)
    gamma = np.asarray(inputs["gamma"], dtype=np.float32)
    beta = np.asarray(inputs["beta"], dtype=np.float32)
    wk = np.asarray(inputs["Wk"], dtype=np.float32)
    wq = np.asarray(inputs["Wq"], dtype=np.float32)
    wv = np.asarray(inputs["Wv"], dtype=np.float32)
    n_batches = int(np.asarray(inputs["n_batches"]))

    n, d = x.shape
    f = wk.shape[0]
    t_seg = n // n_batches
    counts = np.bincount(batch, minlength=n_batches)
    if not (np.all(counts == t_seg) and np.all(np.diff(batch) >= 0)):
        raise NotImplementedError("kernel specialized for equal sorted segments")
    if np.any(beta != 0.0):
        raise NotImplementedError("kernel specialized for beta == 0")

    wkg = (wk * gamma[None, :]).astype(np.float64)
    wvg = (wv * gamma[None, :]).astype(np.float64)
    wqg = (wq * gamma[None, :]).astype(np.float64)
    wkv_t = np.concatenate([wkg, wvg], axis=0).T            # [d, 2f]
    wq_t = wqg.T                                            # [d, f]
    # fold the LN centering into the weights:
    #   x @ (W - 1 s~/d) = (x - mu 1) @ W   since 1 @ W = colsums(W)
    wkv_t = wkv_t - wkv_t.sum(axis=0, keepdims=True) / d
    wq_t = wq_t - wq_t.sum(axis=0, keepdims=True) / d
    ident = np.eye(128, dtype=np.float64)
    wpack = np.ascontiguousarray(
        np.concatenate([wkv_t, wq_t, ident], axis=1).astype(np.float32)
    )

    return x, wpack, n, d, f, n_batches, t_seg


def _run(inputs, trace=False):
    x, wpack, n, d, f, n_batches, t_seg = _prep(inputs)

    segs_per_core = n_batches // N_CORES
    tok_per_core = segs_per_core * t_seg
    nc = _build(tok_per_core, segs_per_core, d, f)

    in_maps = []
    for c in range(N_CORES):
        m = {
            "x": np.ascontiguousarray(x[c * tok_per_core:(c + 1) * tok_per_core]),
            "wpack": wpack,
        }
        in_maps.append(m)

    res = run_bass_kernel_spmd(nc, in_maps, list(range(N_CORES)), trace=trace)
    z = np.concatenate([res.results[c]["z"] for c in range(N_CORES)], axis=0)
    return z, res


def kernel(**inputs) -> np.ndarray:
    z, _ = _run(inputs, trace=False)
    return z
